# revision 1
# baseline (speedup 1.0000x reference)
"""Causal self-attention (RoPE) Trainium2 Bass kernel, 8-way sharded.

Sharding: core c handles batch c//4 and heads 4*(c%4) .. 4*(c%4)+4
(tensor-parallel over heads x data-parallel over batch). Each core
computes its QKV column shard, RoPE, causal attention for its 4 heads,
and a row-shard of the out-projection; the host sums the 4 partial
outputs per batch (the all-reduce realized at gather time).

Per-core kernel layout (everything orientation-chosen to avoid on-chip
transposes):
  - x^T tiles [d,t] arrive pre-transposed from host.
  - Q^T,K^T [hd,t] = w_tile.T @ x^T  (w stationary), RoPE applied via a
    rotate-by-64 permutation-matrix matmul + aligned DVE combine.
  - V [t,hd] = x^T_tile.T @ w_v  (x^T stationary).
  - S^T [k,q] = K_slice.T @ Q_block ; exp on ACT (no max subtraction --
    scores are O(1) bounded); causal mask = multiplicative 0/1 post-exp.
  - softmax denominators via all-ones stationary matmul (broadcasts the
    partition-dim sums to all 128 partitions).
  - attn^T [hd,q] = V_tile.T @ P^T, normalized by reciprocal * mul.
  - out[t,e] = attnT_slice.T @ w_out_rows, accumulated over the 4 heads.
"""

import sys

if "/opt/trn_rl_repo" not in sys.path:
    sys.path.insert(0, "/opt/trn_rl_repo")

import numpy as np
import ml_dtypes

import concourse.bass as bass
import concourse.mybir as mybir
import concourse.tile as tile
from concourse import bacc
from concourse.bass_utils import run_bass_kernel_spmd

FP32 = mybir.dt.float32
BF16 = mybir.dt.bfloat16
BF16_NP = ml_dtypes.bfloat16

B = 2
T = 2048
DIM = 2048
NUM_HEADS = 16
HEAD_DIM = 128
INNER = NUM_HEADS * HEAD_DIM
N_CORES = 8
NH = 4            # heads per core
P = 128           # partitions
TB = T // 512     # 4 t-blocks of 512 tokens
DT = DIM // P     # 16 d-tiles
KT = T // P       # 16 k-tiles of 128 tokens
SCALE = 1.0 / float(np.sqrt(HEAD_DIM))

_CACHE = {}


def _build_nc():
    nc = bacc.Bacc(None, target_bir_lowering=False)

    xt_d = nc.declare_dram_parameter("xt", [TB, P, DT * 512], BF16, isOutput=False)
    wqk_d = nc.declare_dram_parameter("wqk", [P, 8 * DT * P], BF16, isOutput=False)
    wv_d = nc.declare_dram_parameter("wv", [P, DT * 512], BF16, isOutput=False)
    wo_d = nc.declare_dram_parameter("wo", [P, NH * DIM], BF16, isOutput=False)
    cos_d = nc.declare_dram_parameter("cosT", [P, T], BF16, isOutput=False)
    sin_d = nc.declare_dram_parameter("sinT", [P, T], BF16, isOutput=False)
    mask_d = nc.declare_dram_parameter("mask", [P, 4 * 512], BF16, isOutput=False)
    perm_d = nc.declare_dram_parameter("perm", [P, P], BF16, isOutput=False)
    out_d = nc.declare_dram_parameter("out", [T, DIM], FP32, isOutput=True)

    EXP = mybir.ActivationFunctionType.Exp

    with tile.TileContext(nc) as tc:
        with (
            tc.tile_pool(name="const", bufs=1) as cpool,
            tc.tile_pool(name="qkstore", bufs=1) as qkpool,
            tc.tile_pool(name="vstore", bufs=1) as vpool,
            tc.tile_pool(name="xt", bufs=24) as xtpool,
            tc.tile_pool(name="tmp", bufs=3) as tmp,
            tc.tile_pool(name="pt", bufs=6) as ptpool,
            tc.tile_pool(name="ptm", bufs=4) as ptmpool,
            tc.tile_pool(name="attnT", bufs=8) as atpool,
            tc.tile_pool(name="outb", bufs=3) as outpool,
        ):
            # --- constants ---
            wqk = cpool.tile([P, 8 * DT * P], BF16)
            for ct in range(8):
                nc.sync.dma_start(
                    wqk[:, ct * DT * P : (ct + 1) * DT * P],
                    wqk_d[:, ct * DT * P : (ct + 1) * DT * P],
                )
            wv = cpool.tile([P, DT * 512], BF16)
            nc.sync.dma_start(wv[:], wv_d[:])
            wo = cpool.tile([P, NH * DIM], BF16)
            nc.sync.dma_start(wo[:], wo_d[:])
            cosT = cpool.tile([P, T], BF16)
            nc.sync.dma_start(cosT[:], cos_d[:])
            sinT = cpool.tile([P, T], BF16)
            nc.sync.dma_start(sinT[:], sin_d[:])
            mask = cpool.tile([P, 4 * 512], BF16)
            nc.sync.dma_start(mask[:], mask_d[:])
            perm = cpool.tile([P, P], BF16)
            nc.sync.dma_start(perm[:], perm_d[:])
            ones = cpool.tile([P, P], BF16)
            nc.gpsimd.memset(ones[:], 1.0)

            # persistent stores: Q^T,K^T post-rope [hd, T] per (q/k, head);
            # V [t-tile-major, hd] per head
            qkstore = qkpool.tile([P, 8 * T], BF16)   # ct = (q h0..h3, k h0..h3)
            vstore = vpool.tile([P, NH * T], BF16)    # per head: (kt, hd)

            # ---------------- QKV + RoPE phase ----------------
            with (
                tc.tile_pool(name="ps_qk", bufs=2, space="PSUM") as ps_qk,
                tc.tile_pool(name="ps_v", bufs=2, space="PSUM") as ps_v,
                tc.tile_pool(name="ps_rope", bufs=2, space="PSUM") as ps_rope,
            ):
                for tb in range(TB):
                    xt_t = []
                    for dt in range(DT):
                        xt_tile = xtpool.tile([P, 512], BF16, tag="xt")
                        nc.sync.dma_start(
                            xt_tile[:], xt_d[tb, :, dt * 512 : (dt + 1) * 512]
                        )
                        xt_t.append(xt_tile)

                    # Q^T, K^T c-tiles with RoPE
                    for ct in range(8):
                        ps = ps_qk.tile([P, 512], FP32)
                        for dt in range(DT):
                            nc.tensor.matmul(
                                ps[:],
                                wqk[:, (ct * DT + dt) * P : (ct * DT + dt + 1) * P],
                                xt_t[dt][:],
                                start=(dt == 0),
                                stop=(dt == DT - 1),
                            )
                        qsb = tmp.tile([P, 512], BF16, tag="qsb")
                        nc.scalar.copy(qsb[:], ps[:])
                        sw = ps_rope.tile([P, 512], FP32)
                        nc.tensor.matmul(sw[:], perm[:], qsb[:], start=True, stop=True)
                        t1 = tmp.tile([P, 512], FP32, tag="t1")
                        nc.vector.tensor_mul(
                            t1[:], ps[:], cosT[:, tb * 512 : (tb + 1) * 512]
                        )
                        t2 = tmp.tile([P, 512], FP32, tag="t2")
                        nc.vector.tensor_mul(
                            t2[:], sw[:], sinT[:, tb * 512 : (tb + 1) * 512]
                        )
                        nc.vector.tensor_add(
                            qkstore[:, ct * T + tb * 512 : ct * T + (tb + 1) * 512],
                            t1[:],
                            t2[:],
                        )

                    # V tiles [t, c] for 4 heads
                    for s in range(4):
                        psv = ps_v.tile([P, 512], FP32)
                        for dt in range(DT):
                            nc.tensor.matmul(
                                psv[:],
                                xt_t[dt][:, s * P : (s + 1) * P],
                                wv[:, dt * 512 : (dt + 1) * 512],
                                start=(dt == 0),
                                stop=(dt == DT - 1),
                            )
                        kt_idx = tb * 4 + s
                        for h in range(NH):
                            nc.vector.tensor_copy(
                                vstore[:, h * T + kt_idx * P : h * T + (kt_idx + 1) * P],
                                psv[:, h * P : (h + 1) * P],
                            )

            # ---------------- attention + out-proj phase ----------------
            with (
                tc.tile_pool(name="ps_s", bufs=2, space="PSUM") as ps_s,
                tc.tile_pool(name="ps_r", bufs=2, space="PSUM") as ps_r,
                tc.tile_pool(name="ps_o", bufs=2, space="PSUM") as ps_o,
                tc.tile_pool(name="ps_y", bufs=2, space="PSUM") as ps_y,
            ):
                for j in range(TB):
                    at_j = []
                    for h in range(NH):
                        qoff = h * T          # Q^T of head h
                        koff = (NH + h) * T   # K^T of head h
                        n_i = 4 * j + 4
                        o_ps = ps_o.tile([P, 512], FP32)
                        r_ps = ps_r.tile([P, 512], FP32)
                        for i in range(n_i):
                            s_ps = ps_s.tile([P, 512], FP32)
                            nc.tensor.matmul(
                                s_ps[:],
                                qkstore[:, koff + i * P : koff + (i + 1) * P],
                                qkstore[:, qoff + j * 512 : qoff + (j + 1) * 512],
                                start=True,
                                stop=True,
                            )
                            pt = ptpool.tile([P, 512], BF16, tag="pt")
                            nc.scalar.activation(pt[:], s_ps[:], EXP, scale=SCALE)
                            if i >= 4 * j:
                                ptm = ptmpool.tile([P, 512], BF16, tag="ptm")
                                off = i - 4 * j
                                nc.vector.tensor_mul(
                                    ptm[:], pt[:], mask[:, off * 512 : (off + 1) * 512]
                                )
                                pt = ptm
                            nc.tensor.matmul(
                                r_ps[:], ones[:], pt[:],
                                start=(i == 0), stop=(i == n_i - 1),
                            )
                            nc.tensor.matmul(
                                o_ps[:],
                                vstore[:, h * T + i * P : h * T + (i + 1) * P],
                                pt[:],
                                start=(i == 0), stop=(i == n_i - 1),
                            )
                        rc = tmp.tile([P, 512], FP32, tag="rc")
                        nc.vector.reciprocal(rc[:], r_ps[:])
                        at = atpool.tile([P, 512], BF16, tag="at")
                        nc.vector.tensor_mul(at[:], o_ps[:], rc[:])
                        at_j.append(at)

                    for s in range(4):
                        for e in range(4):
                            y_ps = ps_y.tile([P, 512], FP32)
                            for h in range(NH):
                                nc.tensor.matmul(
                                    y_ps[:],
                                    at_j[h][:, s * P : (s + 1) * P],
                                    wo[:, h * DIM + e * 512 : h * DIM + (e + 1) * 512],
                                    start=(h == 0),
                                    stop=(h == NH - 1),
                                )
                            yo = outpool.tile([P, 512], FP32, tag="yo")
                            nc.vector.tensor_copy(yo[:], y_ps[:])
                            t0 = j * 512 + s * P
                            nc.sync.dma_start(
                                out_d[t0 : t0 + P, e * 512 : (e + 1) * 512], yo[:]
                            )

    nc.compile()
    return nc


def _rope_tables():
    inv_freq = 1.0 / (
        10000.0 ** (np.arange(0, HEAD_DIM, 2, dtype=np.float32) / HEAD_DIM)
    )
    t = np.arange(T, dtype=np.float32)
    freqs = np.einsum("i,j->ij", t, inv_freq)          # [T, 64]
    emb = np.concatenate([freqs, freqs], axis=-1)      # [T, 128]
    cosT = np.cos(emb).T.astype(BF16_NP)               # [128, T]
    sinT = np.sin(emb).T                               # [128, T]
    sinS = np.concatenate([-sinT[:64], sinT[64:]], axis=0).astype(BF16_NP)
    return np.ascontiguousarray(cosT), np.ascontiguousarray(sinS)


def kernel(x, w_qkv, w_out):
    x = np.asarray(x, dtype=np.float32)
    w_qkv = np.asarray(w_qkv, dtype=np.float32)
    w_out = np.asarray(w_out, dtype=np.float32)

    cosT, sinS = _rope_tables()

    perm = np.zeros((P, P), dtype=BF16_NP)
    for i in range(P):
        perm[(i + 64) % P, i] = 1

    mask = np.zeros((P, 4 * 512), dtype=BF16_NP)
    r_idx = np.arange(P)[:, None]
    c_idx = np.arange(512)[None, :]
    for oi, off in enumerate((0, 128, 256, 384)):
        mask[:, oi * 512 : (oi + 1) * 512] = (r_idx + off <= c_idx).astype(BF16_NP)

    # per-batch x^T tiles: [TB, 128, DT*512]
    xts = []
    for b in range(B):
        xT = np.ascontiguousarray(x[b].T).astype(BF16_NP)          # [D, T]
        xth = (
            xT.reshape(DT, P, TB, 512).transpose(2, 1, 0, 3).reshape(TB, P, DT * 512)
        )
        xts.append(np.ascontiguousarray(xth))

    # per head-group weight shards
    wqks, wvs, wos = [], [], []
    for g in range(4):
        h0 = NH * g
        cols = [w_qkv[:, 128 * (h0 + h) : 128 * (h0 + h + 1)] for h in range(NH)]
        cols += [
            w_qkv[:, INNER + 128 * (h0 + h) : INNER + 128 * (h0 + h + 1)]
            for h in range(NH)
        ]
        W = np.concatenate(cols, axis=1)                            # [D, 8*128]
        wqk_h = (
            W.reshape(DT, P, 8, P).transpose(1, 2, 0, 3).reshape(P, 8 * DT * P)
        ).astype(BF16_NP)
        wqks.append(np.ascontiguousarray(wqk_h))

        WV = w_qkv[:, 2 * INNER + 128 * h0 : 2 * INNER + 128 * (h0 + NH)]  # [D, 512]
        wv_h = WV.reshape(DT, P, 512).transpose(1, 0, 2).reshape(P, DT * 512)
        wvs.append(np.ascontiguousarray(wv_h.astype(BF16_NP)))

        WO = w_out[128 * h0 : 128 * (h0 + NH), :]                   # [512, D]
        wo_h = WO.reshape(NH, P, DIM).transpose(1, 0, 2).reshape(P, NH * DIM)
        wos.append(np.ascontiguousarray(wo_h.astype(BF16_NP)))

    if "nc" not in _CACHE:
        _CACHE["nc"] = _build_nc()
    nc = _CACHE["nc"]

    in_maps = []
    for c in range(N_CORES):
        b, g = divmod(c, 4)
        in_maps.append(
            {
                "xt": xts[b],
                "wqk": wqks[g],
                "wv": wvs[g],
                "wo": wos[g],
                "cosT": cosT,
                "sinT": sinS,
                "mask": mask,
                "perm": perm,
            }
        )

    res = run_bass_kernel_spmd(nc, in_maps, core_ids=list(range(N_CORES)))

    out = np.zeros((B, T, DIM), dtype=np.float32)
    for c in range(N_CORES):
        b = c // 4
        out[b] += res.results[c]["out"]
    return out


# revision 31
# speedup vs baseline: 20741.2971x; 20741.2971x over previous
"""Causal self-attention (RoPE) Trainium2 Bass kernel, 8-way sharded.

Sharding: core c handles batch c//4 and heads 4*(c%4) .. 4*(c%4)+4
(tensor-parallel over heads x data-parallel over batch). Each core
computes its QKV column shard, RoPE, causal attention for its 4 heads,
and a row-shard of the out-projection; the host sums the 4 partial
outputs per batch (the all-reduce realized at gather time).

Per-core kernel layout (everything orientation-chosen to avoid on-chip
transposes):
  - x^T tiles [d,t] arrive pre-transposed from host.
  - Q^T,K^T [hd,t] = w_tile.T @ x^T  (w stationary), RoPE applied via a
    rotate-by-64 permutation-matrix matmul + aligned DVE combine.
  - V [t,hd] = x^T_tile.T @ w_v  (x^T stationary).
  - S^T [k,q] = K_slice.T @ Q_block ; exp on ACT (no max subtraction --
    scores are O(1) bounded); causal mask = multiplicative 0/1 post-exp.
  - softmax denominators via all-ones stationary matmul (broadcasts the
    partition-dim sums to all 128 partitions).
  - attn^T [hd,q] = V_tile.T @ P^T, normalized by reciprocal * mul.
  - out[t,e] = attnT_slice.T @ w_out_rows, accumulated over the 4 heads.
"""

import sys

if "/opt/trn_rl_repo" not in sys.path:
    sys.path.insert(0, "/opt/trn_rl_repo")

import numpy as np
import ml_dtypes

import concourse.bass as bass
import concourse.mybir as mybir
import concourse.tile as tile
from concourse import bacc
from concourse.bass_utils import run_bass_kernel_spmd

FP32 = mybir.dt.float32
BF16 = mybir.dt.bfloat16
BF16_NP = ml_dtypes.bfloat16

B = 2
T = 2048
DIM = 2048
NUM_HEADS = 16
HEAD_DIM = 128
INNER = NUM_HEADS * HEAD_DIM
N_CORES = 8
NH = 4            # heads per core
P = 128           # partitions
TB = T // 512     # 4 t-blocks of 512 tokens
DT = DIM // P     # 16 d-tiles
KT = T // P       # 16 k-tiles of 128 tokens
SCALE = 1.0 / float(np.sqrt(HEAD_DIM))

_CACHE = {}


def _build_nc(reps=1, opts=None):
    o = {
        "early_dma": True,    # load compute-critical tiles first
        "dt_major": True,     # QKV q/k matmuls dt-outer in groups of 4 c-tiles
        "ps_qk": 4,
        "ps_s": 3,
        "ps_r": 1,
        "ps_o": 2,
        "ps_y": 2,
        # ablation switches (timing accounting only -- break numerics)
        "no_sums": False,
        "no_rope": False,
        # rope via cross-partition DVE ops instead of perm matmul
        "rope_xpart": False,
        "qkv_only": False,
        "no_outproj": False,
        "no_exp": False,
        "spread_outproj": True,
        # sums grouping: DVE-add G pt tiles before each ones-matmul (1 = off)
        "sums_group": 4,
        # attention i-loop software-pipeline depth: S-matmuls emitted this
        # many tiles ahead of their sums/PV consumers, so PE has queued work
        # while ACT computes exp
        "lookahead": 2,
    }
    if opts:
        o.update(opts)
    nc = bacc.Bacc(None, target_bir_lowering=False)

    xt_d = nc.declare_dram_parameter("xt", [TB, P, DT * 512], BF16, isOutput=False)
    wqk_d = nc.declare_dram_parameter("wqk", [P, 8 * DT * P], BF16, isOutput=False)
    wv_d = nc.declare_dram_parameter("wv", [P, DT * 512], BF16, isOutput=False)
    wo_d = nc.declare_dram_parameter("wo", [P, NH * DIM], BF16, isOutput=False)
    cos_d = nc.declare_dram_parameter("cosT", [P, T], BF16, isOutput=False)
    sin_d = nc.declare_dram_parameter("sinT", [P, T], BF16, isOutput=False)
    mask_d = nc.declare_dram_parameter("mask", [P, 4 * 512], BF16, isOutput=False)
    perm_d = nc.declare_dram_parameter("perm", [P, P], BF16, isOutput=False)
    out_d = nc.declare_dram_parameter("out", [T, DIM], FP32, isOutput=True)

    EXP = mybir.ActivationFunctionType.Exp

    with tile.TileContext(nc) as tc:
        with (
            tc.tile_pool(name="const", bufs=1) as cpool,
            tc.tile_pool(name="qkstore", bufs=1) as qkpool,
            tc.tile_pool(name="vstore", bufs=1) as vpool,
            tc.tile_pool(name="xt", bufs=20) as xtpool,
            tc.tile_pool(name="tmp", bufs=3) as tmp,
            tc.tile_pool(name="pt", bufs=10) as ptpool,
            tc.tile_pool(name="ptm", bufs=6) as ptmpool,
            tc.tile_pool(name="attnT", bufs=8) as atpool,
            tc.tile_pool(name="outb", bufs=6) as outpool,
            tc.tile_pool(name="sacc", bufs=3) as saccpool,
        ):
            # --- constants ---
            wqk = cpool.tile([P, 8 * DT * P], BF16)
            wv = cpool.tile([P, DT * 512], BF16)
            wo = cpool.tile([P, NH * DIM], BF16)
            cosT = cpool.tile([P, T], BF16)
            sinT = cpool.tile([P, T], BF16)
            mask = cpool.tile([P, 4 * 512], BF16)
            perm = cpool.tile([P, P], BF16)
            ones = cpool.tile([P, P], BF16)

            def load_wqk_group(grp):
                # one ct-group = 4 c-tiles worth of stationary weight slices
                nc.sync.dma_start(
                    wqk[:, grp * 4 * DT * P : (grp + 1) * 4 * DT * P],
                    wqk_d[:, grp * 4 * DT * P : (grp + 1) * 4 * DT * P],
                )

            xt0 = []
            if o["early_dma"]:
                # compute-critical first: wqk group 0, xt(tb=0), rope tables
                load_wqk_group(0)
                for dt in range(DT):
                    xt_tile = xtpool.tile([P, 512], BF16, tag="xt")
                    nc.sync.dma_start(xt_tile[:], xt_d[0, :, dt * 512 : (dt + 1) * 512])
                    xt0.append(xt_tile)
                nc.sync.dma_start(perm[:], perm_d[:])
                nc.sync.dma_start(cosT[:], cos_d[:])
                nc.sync.dma_start(sinT[:], sin_d[:])
                load_wqk_group(1)
                nc.sync.dma_start(wv[:], wv_d[:])
                nc.sync.dma_start(mask[:], mask_d[:])
                nc.sync.dma_start(wo[:], wo_d[:])
            else:
                load_wqk_group(0)
                load_wqk_group(1)
                nc.sync.dma_start(wv[:], wv_d[:])
                nc.sync.dma_start(wo[:], wo_d[:])
                nc.sync.dma_start(cosT[:], cos_d[:])
                nc.sync.dma_start(sinT[:], sin_d[:])
                nc.sync.dma_start(mask[:], mask_d[:])
                nc.sync.dma_start(perm[:], perm_d[:])
            nc.gpsimd.memset(ones[:], 1.0)

            # persistent stores: Q^T,K^T post-rope [hd, T] per (q/k, head);
            # V [t-tile-major, hd] per head
            qkstore = qkpool.tile([P, 8 * T], BF16)   # ct = (q h0..h3, k h0..h3)
            vstore = vpool.tile([P, NH * T], BF16)    # per head: (kt, hd)

            # ---------------- QKV + RoPE phase ----------------
            for _rep in range(reps):
              with (
                tc.tile_pool(name="ps_qk", bufs=o["ps_qk"], space="PSUM") as ps_qk,
                tc.tile_pool(name="ps_v", bufs=2, space="PSUM") as ps_v,
                tc.tile_pool(name="ps_rope", bufs=2, space="PSUM") as ps_rope,
              ):
                for tb in range(TB):
                    if tb == 0 and o["early_dma"] and _rep == 0:
                        xt_t = xt0
                    else:
                        xt_t = []
                        for dt in range(DT):
                            xt_tile = xtpool.tile([P, 512], BF16, tag="xt")
                            nc.sync.dma_start(
                                xt_tile[:], xt_d[tb, :, dt * 512 : (dt + 1) * 512]
                            )
                            xt_t.append(xt_tile)

                    def rope_and_store(ps, ct):
                        if o["no_rope"]:
                            nc.scalar.copy(
                                qkstore[:, ct * T + tb * 512 : ct * T + (tb + 1) * 512],
                                ps[:],
                            )
                            return
                        t1 = tmp.tile([P, 512], FP32, tag="t1")
                        nc.vector.tensor_mul(
                            t1[:], ps[:], cosT[:, tb * 512 : (tb + 1) * 512]
                        )
                        t2 = tmp.tile([P, 512], FP32, tag="t2")
                        tbs = slice(tb * 512, (tb + 1) * 512)
                        if o["rope_xpart"]:
                            nc.vector.tensor_mul(
                                t2[0:64, :], ps[64:128, :], sinT[0:64, tbs]
                            )
                            nc.vector.tensor_mul(
                                t2[64:128, :], ps[0:64, :], sinT[64:128, tbs]
                            )
                        else:
                            qsb = tmp.tile([P, 512], BF16, tag="qsb")
                            nc.scalar.copy(qsb[:], ps[:])
                            sw = ps_rope.tile([P, 512], FP32)
                            nc.tensor.matmul(
                                sw[:], perm[:], qsb[:], start=True, stop=True
                            )
                            nc.vector.tensor_mul(t2[:], sw[:], sinT[:, tbs])
                        nc.vector.tensor_add(
                            qkstore[:, ct * T + tb * 512 : ct * T + (tb + 1) * 512],
                            t1[:],
                            t2[:],
                        )

                    # Q^T, K^T c-tiles with RoPE
                    if o["dt_major"]:
                        for grp in range(2):
                            pss = [ps_qk.tile([P, 512], FP32, name="psqk", tag="psqk") for _ in range(4)]
                            for dt in range(DT):
                                for ci in range(4):
                                    ct = grp * 4 + ci
                                    nc.tensor.matmul(
                                        pss[ci][:],
                                        wqk[:, (ct * DT + dt) * P : (ct * DT + dt + 1) * P],
                                        xt_t[dt][:],
                                        start=(dt == 0),
                                        stop=(dt == DT - 1),
                                    )
                            for ci in range(4):
                                rope_and_store(pss[ci], grp * 4 + ci)
                    else:
                        for ct in range(8):
                            ps = ps_qk.tile([P, 512], FP32)
                            for dt in range(DT):
                                nc.tensor.matmul(
                                    ps[:],
                                    wqk[:, (ct * DT + dt) * P : (ct * DT + dt + 1) * P],
                                    xt_t[dt][:],
                                    start=(dt == 0),
                                    stop=(dt == DT - 1),
                                )
                            rope_and_store(ps, ct)

                    # V tiles [t, c] for 4 heads
                    for s in range(4):
                        psv = ps_v.tile([P, 512], FP32)
                        for dt in range(DT):
                            nc.tensor.matmul(
                                psv[:],
                                xt_t[dt][:, s * P : (s + 1) * P],
                                wv[:, dt * 512 : (dt + 1) * 512],
                                start=(dt == 0),
                                stop=(dt == DT - 1),
                            )
                        kt_idx = tb * 4 + s
                        for h in range(NH):
                            nc.vector.tensor_copy(
                                vstore[:, h * T + kt_idx * P : h * T + (kt_idx + 1) * P],
                                psv[:, h * P : (h + 1) * P],
                            )

              # ---------------- attention + out-proj phase ----------------
              if o["qkv_only"]:
                  continue
              with (
                tc.tile_pool(name="ps_s", bufs=o["ps_s"], space="PSUM") as ps_s,
                tc.tile_pool(name="ps_r", bufs=o["ps_r"], space="PSUM") as ps_r,
                tc.tile_pool(name="ps_o", bufs=o["ps_o"], space="PSUM") as ps_o,
                tc.tile_pool(name="ps_y", bufs=o["ps_y"], space="PSUM") as ps_y,
              ):
                # Global software pipeline across (j, h, i): the S-matmul/exp
                # producer cursor runs `lookahead` stages ahead of the
                # sums/PV consumer cursor, so PE always has independent
                # S-matmuls queued while ACT computes exp. Out-proj emits as
                # soon as its j's consumers have drained, filling PE while
                # ACT works on the next j's exps.
                LA = o["lookahead"]
                pts = {}       # (j,h,i) -> pt tile
                ros = {}       # (j,h) -> (r_ps, o_ps)
                at_tiles = {}  # (j,h) -> at

                stages = [
                    (j, h, i)
                    for j in range(TB)
                    for h in range(NH)
                    for i in range(4 * j + 4)
                ]

                def emit_s(key):
                    j, h, i = key
                    qoff = h * T
                    koff = (NH + h) * T
                    s_ps = ps_s.tile([P, 512], FP32, name="s_ps", tag="s_ps")
                    nc.tensor.matmul(
                        s_ps[:],
                        qkstore[:, koff + i * P : koff + (i + 1) * P],
                        qkstore[:, qoff + j * 512 : qoff + (j + 1) * 512],
                        start=True,
                        stop=True,
                    )
                    if o["no_exp"]:
                        pts[key] = mask[:, 0:512]
                        return
                    pt = ptpool.tile([P, 512], BF16, name="pt", tag="pt")
                    nc.scalar.activation(pt[:], s_ps[:], EXP, scale=SCALE)
                    if i >= 4 * j:
                        ptm = ptmpool.tile([P, 512], BF16, name="ptm", tag="ptm")
                        off = i - 4 * j
                        nc.vector.tensor_mul(
                            ptm[:], pt[:], mask[:, off * 512 : (off + 1) * 512]
                        )
                        pt = ptm
                    pts[key] = pt

                sum_pend = {}

                def emit_consume(key):
                    j, h, i = key
                    n_i = 4 * j + 4
                    G = o["sums_group"]
                    pt = pts.pop(key)
                    if i == 0:
                        o_ps = ps_o.tile([P, 512], FP32, name="o_ps", tag="o_ps")
                        r_ps = (
                            None
                            if o["no_sums"]
                            else ps_r.tile([P, 512], FP32, name="r_ps", tag="r_ps")
                        )
                        ros[(j, h)] = (r_ps, o_ps)
                        sum_pend[(j, h)] = ([], [0])
                    r_ps, o_ps = ros[(j, h)]
                    nc.tensor.matmul(
                        o_ps[:],
                        vstore[:, h * T + i * P : h * T + (i + 1) * P],
                        pt[:],
                        start=(i == 0), stop=(i == n_i - 1),
                    )
                    if not o["no_sums"]:
                        pend, gidx = sum_pend[(j, h)]
                        pend.append(pt)
                        if len(pend) == G or i == n_i - 1:
                            if len(pend) == 1:
                                rhs = pend[0]
                            else:
                                acc = saccpool.tile(
                                    [P, 512], BF16, name="sacc", tag="sacc"
                                )
                                nc.vector.tensor_add(acc[:], pend[0][:], pend[1][:])
                                for extra in pend[2:]:
                                    nc.vector.tensor_add(acc[:], acc[:], extra[:])
                                rhs = acc
                            nc.tensor.matmul(
                                r_ps[:], ones[:], rhs[:],
                                start=(gidx[0] == 0), stop=(i == n_i - 1),
                            )
                            pend.clear()
                            gidx[0] += 1
                    if i == n_i - 1:
                        r_ps, o_ps = ros.pop((j, h))
                        at = atpool.tile([P, 512], BF16, name="at", tag="at")
                        if o["no_sums"]:
                            nc.vector.tensor_copy(at[:], o_ps[:])
                        else:
                            rc = tmp.tile([P, 512], FP32, tag="rc")
                            nc.vector.reciprocal(rc[:], r_ps[:])
                            nc.vector.tensor_mul(at[:], o_ps[:], rc[:])
                        at_tiles[(j, h)] = at
                        if h == NH - 1:
                            if o["no_outproj"]:
                                for hh in range(NH):
                                    at_tiles.pop((j, hh))
                            else:
                                emit_outproj(j)

                y_pend = []

                def emit_y_group(j, at_j, s, e):
                    y_ps = ps_y.tile([P, 512], FP32, name="y_ps", tag="y_ps")
                    for h in range(NH):
                        nc.tensor.matmul(
                            y_ps[:],
                            at_j[h][:, s * P : (s + 1) * P],
                            wo[:, h * DIM + e * 512 : h * DIM + (e + 1) * 512],
                            start=(h == 0),
                            stop=(h == NH - 1),
                        )
                    yo = outpool.tile([P, 512], FP32, tag="yo")
                    nc.vector.tensor_copy(yo[:], y_ps[:])
                    t0 = j * 512 + s * P
                    nc.sync.dma_start(
                        out_d[t0 : t0 + P, e * 512 : (e + 1) * 512], yo[:]
                    )

                def emit_outproj(j):
                    at_j = [at_tiles.pop((j, h)) for h in range(NH)]
                    groups = [(j, at_j, s, e) for s in range(4) for e in range(4)]
                    if o["spread_outproj"]:
                        y_pend.extend(groups)
                    else:
                        for g in groups:
                            emit_y_group(*g)

                for k in range(len(stages) + LA):
                    if k < len(stages):
                        emit_s(stages[k])
                    if k - LA >= 0:
                        emit_consume(stages[k - LA])
                    if y_pend:
                        emit_y_group(*y_pend.pop(0))
                while y_pend:
                    emit_y_group(*y_pend.pop(0))

    nc.compile()
    return nc


def _rope_tables():
    inv_freq = 1.0 / (
        10000.0 ** (np.arange(0, HEAD_DIM, 2, dtype=np.float32) / HEAD_DIM)
    )
    t = np.arange(T, dtype=np.float32)
    freqs = np.einsum("i,j->ij", t, inv_freq)          # [T, 64]
    emb = np.concatenate([freqs, freqs], axis=-1)      # [T, 128]
    cosT = np.cos(emb).T.astype(BF16_NP)               # [128, T]
    sinT = np.sin(emb).T                               # [128, T]
    sinS = np.concatenate([-sinT[:64], sinT[64:]], axis=0).astype(BF16_NP)
    return np.ascontiguousarray(cosT), np.ascontiguousarray(sinS)


def kernel(x, w_qkv, w_out):
    x = np.asarray(x, dtype=np.float32)
    w_qkv = np.asarray(w_qkv, dtype=np.float32)
    w_out = np.asarray(w_out, dtype=np.float32)

    cosT, sinS = _rope_tables()

    perm = np.zeros((P, P), dtype=BF16_NP)
    for i in range(P):
        perm[(i + 64) % P, i] = 1

    mask = np.zeros((P, 4 * 512), dtype=BF16_NP)
    r_idx = np.arange(P)[:, None]
    c_idx = np.arange(512)[None, :]
    for oi, off in enumerate((0, 128, 256, 384)):
        mask[:, oi * 512 : (oi + 1) * 512] = (r_idx + off <= c_idx).astype(BF16_NP)

    # per-batch x^T tiles: [TB, 128, DT*512]
    xts = []
    for b in range(B):
        xT = np.ascontiguousarray(x[b].T).astype(BF16_NP)          # [D, T]
        xth = (
            xT.reshape(DT, P, TB, 512).transpose(2, 1, 0, 3).reshape(TB, P, DT * 512)
        )
        xts.append(np.ascontiguousarray(xth))

    # per head-group weight shards
    wqks, wvs, wos = [], [], []
    for g in range(4):
        h0 = NH * g
        cols = [w_qkv[:, 128 * (h0 + h) : 128 * (h0 + h + 1)] for h in range(NH)]
        cols += [
            w_qkv[:, INNER + 128 * (h0 + h) : INNER + 128 * (h0 + h + 1)]
            for h in range(NH)
        ]
        W = np.concatenate(cols, axis=1)                            # [D, 8*128]
        wqk_h = (
            W.reshape(DT, P, 8, P).transpose(1, 2, 0, 3).reshape(P, 8 * DT * P)
        ).astype(BF16_NP)
        wqks.append(np.ascontiguousarray(wqk_h))

        WV = w_qkv[:, 2 * INNER + 128 * h0 : 2 * INNER + 128 * (h0 + NH)]  # [D, 512]
        wv_h = WV.reshape(DT, P, 512).transpose(1, 0, 2).reshape(P, DT * 512)
        wvs.append(np.ascontiguousarray(wv_h.astype(BF16_NP)))

        WO = w_out[128 * h0 : 128 * (h0 + NH), :]                   # [512, D]
        wo_h = WO.reshape(NH, P, DIM).transpose(1, 0, 2).reshape(P, NH * DIM)
        wos.append(np.ascontiguousarray(wo_h.astype(BF16_NP)))

    if "nc" not in _CACHE:
        _CACHE["nc"] = _build_nc()
    nc = _CACHE["nc"]

    in_maps = []
    for c in range(N_CORES):
        b, g = divmod(c, 4)
        in_maps.append(
            {
                "xt": xts[b],
                "wqk": wqks[g],
                "wv": wvs[g],
                "wo": wos[g],
                "cosT": cosT,
                "sinT": sinS,
                "mask": mask,
                "perm": perm,
            }
        )

    res = run_bass_kernel_spmd(nc, in_maps, core_ids=list(range(N_CORES)))

    out = np.zeros((B, T, DIM), dtype=np.float32)
    for c in range(N_CORES):
        b = c // 4
        out[b] += res.results[c]["out"]
    return out
